# revision 2
# baseline (speedup 1.0000x reference)
"""AttentionPooler Trainium2 kernel.

8-core data-parallel over batch (4 batches/core). Single pass over the large
encoder_outputs tensor (converted to fp16 on the host — halves HBM traffic;
all on-chip matmuls run fp16 at 1 cycle/row) with the small weights
algebraically folded on the host:

  scores[s,j] = x[s,:] @ Ac            Ac = column-centered gamma*q~^T/8
                                       (column-centering applies the
                                        LayerNorm mean subtraction exactly)
  es'[s,j] = exp(r_s*scores + ln r_s)  = r_s * exp(r_s*scores)
                                       (rstd folded into the exp bias, so
                                        the U matmul consumes RAW x — no
                                        768-wide x*r multiply on DVE)
  U[j,:]   = sum_s es'[s,j] * [x[s,:], mu_s, 1/r_s]   (PSUM accumulated)
  pooled   = (U[:, :768] - c1) / l     c1 = sum es' mu, l = sum es' / r = sum es
  ctx_h    = pooled_h @ (gamma*Wv)_h   per-head [32,768]@[768,64]
  out      = ctx @ Wo + beta@Wv@Wo

All PE transposes are replaced by XBAR DMA transposes (fp16-only HW path):
x^T comes straight from DRAM, pooled^T from SBUF.
"""
import numpy as np

import concourse.bass as bass
import concourse.bacc as bacc
import concourse.tile as tile
from concourse import mybir
from concourse.bass_utils import run_bass_kernel_spmd

# ---- problem constants (hardcoded per harness contract) ----
B, S, DIM = 32, 4096, 768
H, NQ, DH = 12, 32, 64
INNER = H * DH          # 768
J = H * NQ              # 384
N_CORES = 8
B_LOC = B // N_CORES    # 4
CHUNK = 128
N_CHUNKS = S // CHUNK   # 32
ET = DIM // 128         # 6 e-tiles of the model dim
JT = J // 128           # 3 j-tiles
EPS = 1e-5

F32 = mybir.dt.float32
F16 = mybir.dt.float16
AF = mybir.ActivationFunctionType
ALU = mybir.AluOpType


def _steer_act_tables(arch: str):
    """Make the act-table-load pass serve Exp from the set that also holds
    Ln, so a kernel alternating Ln/Exp loads tables exactly once."""
    from concourse.hw_specs import get_activation_tables

    tables = get_activation_tables(arch)  # functools.cache -> shared dict
    keep = "natural_log_exp_and_others"
    if keep in tables:
        for name, funcs in tables.items():
            if name != keep:
                funcs.discard(AF.Exp)


def _build_program():
    nc = bacc.Bacc(
        "TRN2", target_bir_lowering=False, debug=False, num_devices=N_CORES
    )
    _steer_act_tables(nc.m.arch)
    x_d = nc.dram_tensor("x", [B_LOC, S, DIM], F16, kind="ExternalInput")
    ac_d = nc.dram_tensor("ac", [128, ET, J], F16, kind="ExternalInput")
    wv_d = nc.dram_tensor("wv", [128, ET, INNER], F16, kind="ExternalInput")
    wo_d = nc.dram_tensor("wo", [128, ET, DIM], F16, kind="ExternalInput")
    y_d = nc.dram_tensor("y", [B_LOC, NQ, DIM], F32, kind="ExternalOutput")

    with tile.TileContext(nc) as tc, \
         tc.tile_pool(name="const", bufs=1) as const, \
         tc.tile_pool(name="xin", bufs=6) as xin, \
         tc.tile_pool(name="work", bufs=5) as work, \
         tc.tile_pool(name="stat", bufs=8) as stat, \
         tc.tile_pool(name="epi", bufs=2) as epi, \
         tc.tile_pool(name="pu", bufs=1, space="PSUM") as pu, \
         tc.tile_pool(name="pt", bufs=3, space="PSUM") as pt:

        ac_sb = const.tile([128, ET, J], F16, tag="ac")
        # wv/wo are first needed ~70us in (first epilogue); issue their DMAs
        # a few chunks into batch 0 so the first x chunks aren't queued
        # behind the weights.
        wv_sb = const.tile([128, ET, INNER], F16, tag="wv")
        wo_sb = const.tile([128, ET, DIM], F16, tag="wo")
        eps_sb = const.tile([128, 1], F32, tag="eps")
        nc.vector.memset(eps_sb[:], EPS)

        TOT = B_LOC * N_CHUNKS
        u_tiles = {}
        stage_state = {}
        ep_state = {}

        def stage_a(gi):
            """DMA (straight + XBAR-transposed) + LN stats for flat chunk gi."""
            b, c = divmod(gi, N_CHUNKS)
            src = x_d[b, c * 128:(c + 1) * 128, :]
            x_t = xin.tile([128, DIM], F16, tag="x", name=f"x_{gi}")
            nc.sync.dma_start(x_t[:], src)
            xT = xin.tile([128, ET, CHUNK], F16, tag="xT", name=f"xT_{gi}")
            nc.sync.dma_start_transpose(xT[:], src)
            if gi == 0:
                # ac rides the HWDGE FIFO right behind chunk 0's data: the
                # scores (one stage later) get their e-tiles just in time.
                nc.sync.dma_start(ac_sb[:], ac_d[:])
            if gi == 8:
                nc.sync.dma_start(wv_sb[:], wv_d[:])
            if gi == 10:
                nc.sync.dma_start(wo_sb[:], wo_d[:])

            st = stat.tile([128, 2, 6], F32, tag="st", name=f"st_{gi}")
            xg = x_t[:].rearrange("p (n f) -> p n f", f=384)
            for g in range(2):
                nc.vector.bn_stats(st[:, g, :], xg[:, g, :])
            mv = stat.tile([128, 2], F16, tag="mv", name=f"mv_{gi}")
            nc.vector.bn_aggr(mv[:], st[:])
            # r = (var+eps)^-1/2 = exp(-0.5*ln(var+eps)); Ln+Exp share an ACT
            # table set (Rsqrt activation is banned for accuracy).
            lnv = stat.tile([128, 1], F32, tag="lnv", name=f"lnv_{gi}")
            nc.scalar.activation(lnv[:], mv[:, 1:2], AF.Ln,
                                 bias=eps_sb[:], scale=1.0)
            r_t = stat.tile([128, 1], F32, tag="r", name=f"r_{gi}")
            nc.scalar.activation(r_t[:], lnv[:], AF.Exp, scale=-0.5)
            # exp bias ln(r) = -0.5*lnv folds the rstd scale into es itself
            nb = stat.tile([128, 1], F32, tag="nb", name=f"nb_{gi}")
            nc.vector.tensor_scalar_mul(nb[:], lnv[:], -0.5)
            # overwrite the var slot with 1/r: mv becomes [mu, 1/r] -- the
            # two trailing U columns (c1 = sum es' mu, l = sum es'/r = sum es)
            nc.scalar.activation(mv[:, 1:2], lnv[:], AF.Exp, scale=0.5)
            stage_state[gi] = (x_t, xT, mv, r_t, nb)

        def stage_b1(gi):
            """scores + exp (U-MMs deferred one more stage so the static PE
            order never waits on the exp ACT latency)."""
            x_t, xT, mv, r_t, nb = stage_state.pop(gi)
            sc = pt.tile([128, J], F32, tag="tp", name=f"sc_{gi}")
            for et in range(ET):
                nc.tensor.matmul(
                    sc[:],
                    xT[:, et, :],
                    ac_sb[:, et, :],
                    start=(et == 0), stop=(et == ET - 1),
                )
            es = work.tile([128, J], F16, tag="es", name=f"es_{gi}")
            nc.scalar.activation(es[:], sc[:], AF.Exp, bias=nb[:], scale=r_t[:])
            stage_state[("v", gi)] = (x_t, mv, es)

        def stage_b2(gi):
            """U accumulation for flat chunk gi."""
            b, c = divmod(gi, N_CHUNKS)
            x_t, mv, es = stage_state.pop(("v", gi))
            if c == 0:
                u_tiles[b] = (
                    [pu.tile([128, 512], F32, tag=f"u{jt}", name=f"u{jt}_{b}")
                     for jt in range(JT)],
                    pu.tile([128, 512], F32, tag="uhiA", name=f"uhiA_{b}"),
                    pu.tile([128, 512], F32, tag="uhiB", name=f"uhiB_{b}"),
                )
            ulo, uhiA, uhiB = u_tiles[b]
            # start=True clears has_written for a whole PSUM bank, so in each
            # shared bank only the first-emitted matmul of chunk 0 carries
            # start=True; later first-writes land as overwrites on cleared
            # bits (start=False).
            last = (c == N_CHUNKS - 1)
            for jt in range(JT):
                nc.tensor.matmul(
                    ulo[jt][:],
                    es[:, jt * 128:(jt + 1) * 128], x_t[:, 0:512],
                    start=(c == 0), stop=last, skip_group_check=True,
                )
            for jt in range(JT):
                dst = (uhiA[:, jt * 256:(jt + 1) * 256] if jt < 2
                       else uhiB[:, 0:256])
                nc.tensor.matmul(
                    dst,
                    es[:, jt * 128:(jt + 1) * 128], x_t[:, 512:768],
                    start=(c == 0 and jt != 1), stop=last,
                    skip_group_check=True,
                )
            for jt in range(JT):
                nc.tensor.matmul(
                    uhiB[:, 256 + 2 * jt:258 + 2 * jt],
                    es[:, jt * 128:(jt + 1) * 128], mv[:],
                    start=False, stop=last, skip_group_check=True,
                )

        def ep1(b):
            """pooled = (U - c1)/l evacuation (DVE/ACT only, frees U banks)."""
            ulo, uhiA, uhiB = u_tiles[b]
            p2 = epi.tile([128, JT, DIM], F16, tag="p2", name=f"p2_{b}")
            for jt in range(JT):
                rl = stat.tile([128, 1], F32, tag="rl", name=f"rl_{b}_{jt}")
                nc.vector.reciprocal(rl[:], uhiB[:, 257 + 2 * jt:258 + 2 * jt])
                cc = stat.tile([128, 1], F32, tag="cc", name=f"cc_{b}_{jt}")
                nc.scalar.copy(cc[:], uhiB[:, 256 + 2 * jt:257 + 2 * jt])
                if jt == 0:
                    # ACT path: Identity(rl*U + (-rl*c1)) == rl*(U - c1)
                    nb = stat.tile([128, 1], F32, tag="nbe", name=f"nbe_{b}")
                    nc.vector.tensor_scalar(
                        out=nb[:], in0=cc[:], scalar1=-1.0, scalar2=rl[:],
                        op0=ALU.mult, op1=ALU.mult,
                    )
                    nc.scalar.activation(
                        p2[:, jt, 0:512], ulo[jt][:],
                        AF.Identity, bias=nb[:], scale=rl[:],
                    )
                    nc.scalar.activation(
                        p2[:, jt, 512:768], uhiA[:, jt * 256:(jt + 1) * 256],
                        AF.Identity, bias=nb[:], scale=rl[:],
                    )
                    continue
                nc.vector.tensor_scalar(
                    out=p2[:, jt, 0:512], in0=ulo[jt][:],
                    scalar1=cc[:], scalar2=rl[:],
                    op0=ALU.subtract, op1=ALU.mult,
                )
                nc.vector.tensor_scalar(
                    out=p2[:, jt, 512:768],
                    in0=(uhiA[:, jt * 256:(jt + 1) * 256] if jt < 2
                         else uhiB[:, 0:256]),
                    scalar1=cc[:], scalar2=rl[:],
                    op0=ALU.subtract, op1=ALU.mult,
                )
            ep_state[b] = p2

        def ep2(b):
            """XBAR-transpose pooled -> p2T[e_local, et, j] (no PE work)."""
            p2 = ep_state.pop(b)
            p2T = epi.tile([128, ET, J], F16, tag="p2T", name=f"p2T_{b}")
            for jt in range(JT):
                nc.sync.dma_start_transpose(
                    p2T[:, :, jt * 128:(jt + 1) * 128], p2[:, jt, :]
                )
            ep_state[b] = p2T

        def ep3(b):
            """ctx = pooled_h @ Wv'_h (pre-transposed), then out = ctx @ Wo."""
            p2T = ep_state.pop(b)
            ctxT = epi.tile([128, ET, NQ], F16, tag="ctxT", name=f"ctxT_{b}")
            for h in range(H):
                cp = pt.tile([64, NQ], F32, tag="tp", name=f"cp_{b}_h{h}")
                for et in range(ET):
                    nc.tensor.matmul(
                        cp[:],
                        wv_sb[:, et, h * 64:(h + 1) * 64],
                        p2T[:, et, h * NQ:(h + 1) * NQ],
                        start=(et == 0), stop=(et == ET - 1),
                    )
                h2 = h % 2
                dst = ctxT[h2 * 64:(h2 + 1) * 64, h // 2, :]
                if h % 2 == 0:
                    nc.scalar.copy(dst, cp[:])
                else:
                    nc.vector.tensor_copy(dst, cp[:])

            oc = epi.tile([NQ, DIM], F32, tag="oc", name=f"oc_{b}")
            for half in range(2):
                po = pt.tile([128, 384], F32, tag="tp", name=f"po_{b}_{half}")
                for g2 in range(ET):
                    nc.tensor.matmul(
                        po[0:NQ, :],
                        ctxT[:, g2, :],
                        wo_sb[:, g2, half * 384:(half + 1) * 384],
                        start=(g2 == 0), stop=(g2 == ET - 1),
                    )
                nc.scalar.copy(oc[:, half * 384:(half + 1) * 384], po[0:NQ, :])
            nc.sync.dma_start(y_d[b], oc[:])

        for gi in range(TOT + 5):
            if gi < TOT:
                stage_a(gi)
            if 1 <= gi <= TOT:
                stage_b1(gi - 1)
            if 2 <= gi <= TOT + 1:
                stage_b2(gi - 2)
            # epilogue pieces trail each batch's last stage_b2 by 0/1/2
            # iterations so their serial chains hide behind the next batch's
            # chunk work.
            for b in range(B_LOC):
                fin = (b + 1) * N_CHUNKS + 1   # gi at which stage_b2(b, last)
                if gi == fin:
                    ep1(b)
                elif gi == fin + 1:
                    ep2(b)
                elif gi == fin + 2:
                    ep3(b)

    nc.compile()
    return nc


_NC_CACHE = None


def _get_program():
    global _NC_CACHE
    if _NC_CACHE is None:
        _NC_CACHE = _build_program()
    return _NC_CACHE


def _fold_weights(queries, Wq, Wkv, Wo, gamma, beta):
    """Host-side algebraic folding of the small weights (all fp32 numpy)."""
    q = queries.astype(np.float64) @ Wq.astype(np.float64)       # [32, 768]
    qh = q.reshape(NQ, H, DH)
    Wk = Wkv[:, :INNER].astype(np.float64)
    Wv = Wkv[:, INNER:].astype(np.float64)
    Wk_h = Wk.reshape(DIM, H, DH)
    # q~[j=(h,n), e] with j head-major
    qt = np.einsum("nhd,ehd->hne", qh, Wk_h, optimize=True).reshape(J, DIM)
    A = (gamma.astype(np.float64)[:, None] * qt.T) / (DH ** 0.5)  # [768, 384]
    Ac = A - A.mean(axis=0, keepdims=True)
    Wvp = gamma.astype(np.float64)[:, None] * Wv                  # [768, 768]
    bvwo = (beta.astype(np.float64) @ Wv) @ Wo.astype(np.float64)  # [768]

    def tile6(m):  # [768, F] -> [128, 6, F] e-tile-major layout
        return np.ascontiguousarray(
            m.reshape(ET, 128, -1).transpose(1, 0, 2)
        ).astype(np.float16)

    return (
        tile6(Ac),
        tile6(Wvp),
        tile6(Wo.astype(np.float64)),
        bvwo.astype(np.float32),
    )


def kernel(encoder_outputs, queries, Wq, Wkv, Wo, ln_gamma, ln_beta):
    x = np.ascontiguousarray(
        np.asarray(encoder_outputs, dtype=np.float32).astype(np.float16)
    )
    queries = np.asarray(queries, dtype=np.float32)
    Wq = np.asarray(Wq, dtype=np.float32)
    Wkv = np.asarray(Wkv, dtype=np.float32)
    Wo_np = np.asarray(Wo, dtype=np.float32)
    gamma = np.asarray(ln_gamma, dtype=np.float32)
    beta = np.asarray(ln_beta, dtype=np.float32)

    ac_t, wv_t, wo_t, bvwo = _fold_weights(queries, Wq, Wkv, Wo_np, gamma, beta)

    nc = _get_program()
    in_maps = [
        {
            "x": x[c * B_LOC:(c + 1) * B_LOC],
            "ac": ac_t,
            "wv": wv_t,
            "wo": wo_t,
        }
        for c in range(N_CORES)
    ]
    res = run_bass_kernel_spmd(nc, in_maps, list(range(N_CORES)))
    y = np.concatenate([res.results[c]["y"] for c in range(N_CORES)], axis=0)
    return (y + bvwo[None, None, :]).astype(np.float32)


# revision 35
# speedup vs baseline: 2.0841x; 2.0841x over previous
"""AttentionPooler Trainium2 kernel.

8-core data-parallel over batch (4 batches/core). Single pass over the large
encoder_outputs tensor (converted to fp16 on the host — halves HBM traffic;
all on-chip matmuls run fp16 at 1 cycle/row) with the small weights
algebraically folded on the host:

  scores[s,j] = x[s,:] @ Ac            Ac = column-centered gamma*q~^T/8
                                       (column-centering applies the
                                        LayerNorm mean subtraction exactly)
  es'[s,j] = exp(r_s*scores + ln r_s)  = r_s * exp(r_s*scores)
                                       (rstd folded into the exp bias, so
                                        the U matmul consumes RAW x — no
                                        768-wide x*r multiply on DVE)
  U[j,:]   = sum_s es'[s,j] * [x[s,:], mu_s, 1/r_s]   (PSUM accumulated)
  pooled   = (U[:, :768] - c1) / l     c1 = sum es' mu, l = sum es' / r = sum es
  ctx_h    = pooled_h @ (gamma*Wv)_h   per-head [32,768]@[768,64]
  out      = ctx @ Wo + beta@Wv@Wo

All PE transposes are replaced by XBAR DMA transposes (fp16-only HW path):
x^T comes straight from DRAM, pooled^T from SBUF.
"""
import numpy as np

import concourse.bass as bass
import concourse.bacc as bacc
import concourse.tile as tile
from concourse import mybir
from concourse.bass_utils import run_bass_kernel_spmd

# ---- problem constants (hardcoded per harness contract) ----
B, S, DIM = 32, 4096, 768
H, NQ, DH = 12, 32, 64
INNER = H * DH          # 768
J = H * NQ              # 384
N_CORES = 8
B_LOC = B // N_CORES    # 4
CHUNK = 128
N_CHUNKS = S // CHUNK   # 32
ET = DIM // 128         # 6 e-tiles of the model dim
JT = J // 128           # 3 j-tiles
EPS = 1e-5

F32 = mybir.dt.float32
F16 = mybir.dt.float16
AF = mybir.ActivationFunctionType
ALU = mybir.AluOpType


def _steer_act_tables(arch: str):
    """Make the act-table-load pass serve Exp from the set that also holds
    Ln, so a kernel alternating Ln/Exp loads tables exactly once."""
    from concourse.hw_specs import get_activation_tables

    tables = get_activation_tables(arch)  # functools.cache -> shared dict
    keep = "natural_log_exp_and_others"
    if keep in tables:
        for name, funcs in tables.items():
            if name != keep:
                funcs.discard(AF.Exp)


def _build_program():
    nc = bacc.Bacc(
        "TRN2", target_bir_lowering=False, debug=False, num_devices=N_CORES
    )
    _steer_act_tables(nc.m.arch)
    x_d = nc.dram_tensor("x", [B_LOC, S, DIM], F16, kind="ExternalInput")
    ac_d = nc.dram_tensor("ac", [128, ET, J], F16, kind="ExternalInput")
    wv_d = nc.dram_tensor("wv", [128, ET, INNER], F16, kind="ExternalInput")
    wo_d = nc.dram_tensor("wo", [128, ET, DIM], F16, kind="ExternalInput")
    id_d = nc.dram_tensor("ident", [128, 128], F16, kind="ExternalInput")
    y_d = nc.dram_tensor("y", [B_LOC, ET, 128, NQ], F32, kind="ExternalOutput")

    with tile.TileContext(nc) as tc, \
         tc.tile_pool(name="const", bufs=1) as const, \
         tc.tile_pool(name="xin", bufs=4) as xin, \
         tc.tile_pool(name="work", bufs=5) as work, \
         tc.tile_pool(name="stat", bufs=8) as stat, \
         tc.tile_pool(name="epi", bufs=2) as epi, \
         tc.tile_pool(name="pu", bufs=1, space="PSUM") as pu, \
         tc.tile_pool(name="pt", bufs=3, space="PSUM") as pt:

        ac_sb = const.tile([128, ET, J], F16, tag="ac")
        # wv/wo are first needed ~70us in (first epilogue); issue their DMAs
        # a few chunks into batch 0 so the first x chunks aren't queued
        # behind the weights.
        wv_sb = const.tile([128, ET, INNER], F16, tag="wv")
        wo_sb = const.tile([128, ET, DIM], F16, tag="wo")
        eps_sb = const.tile([128, 1], F32, tag="eps")
        nc.vector.memset(eps_sb[:], EPS)
        id_sb = const.tile([128, 128], F16, tag="ident")
        nc.sync.dma_start(id_sb[:], id_d[:])

        # ac first on the SP ring: the first scores need it ~6us in
        nc.sync.dma_start(ac_sb[:], ac_d[:])

        TOT = B_LOC * N_CHUNKS
        u_tiles = {}
        stage_state = {}
        ep_state = {}

        GRAN = 4                       # chunks per DMA block (512 seq rows)
        NB = TOT // GRAN               # 32 blocks/core, 8 per batch

        def stage_a(bi):
            """Block DMA issue: one straight load (SP ring) + one XBAR
            transpose (ACT ring) covering GRAN chunks = 512 seq rows."""
            b, c0 = divmod(bi * GRAN, N_CHUNKS)
            src = x_d[b, c0 * 128:(c0 + GRAN) * 128, :]
            x_t = xin.tile([128, GRAN, DIM], F16, tag="x", name=f"x_{bi}")
            if bi == 0:
                for k in range(GRAN):
                    nc.sync.dma_start(
                        x_t[:, k, :], x_d[b, k * 128:(k + 1) * 128, :]
                    )
            else:
                nc.sync.dma_start(
                    x_t[:], src.rearrange("(sub p) e -> p sub e", p=128)
                )
            if bi == 2:
                nc.sync.dma_start(wv_sb[:], wv_d[:])
            if bi == 3:
                nc.sync.dma_start(wo_sb[:], wo_d[:])
            stage_state[("d", bi)] = x_t

        def stage_st(bi):
            """LayerNorm row stats for all GRAN sub-chunks of block bi."""
            x_t = stage_state.pop(("d", bi))
            st = stat.tile([128, 2 * GRAN, 6], F32, tag="st", name=f"st_{bi}")
            xg = x_t[:].rearrange("p sub (n f) -> p (sub n) f", f=384)
            for g in range(2 * GRAN):
                nc.vector.bn_stats(st[:, g, :], xg[:, g, :])
            mv = stat.tile([128, GRAN, 2], F16, tag="mv", name=f"mv_{bi}")
            for k in range(GRAN):
                nc.vector.bn_aggr(mv[:, k, :], st[:, 2 * k:2 * k + 2, :])
            # r = (var+eps)^-1/2 = exp(-0.5*ln(var+eps)); Ln+Exp share an ACT
            # table set (Rsqrt activation is banned for accuracy).
            lnv = stat.tile([128, GRAN], F32, tag="lnv", name=f"lnv_{bi}")
            nc.scalar.activation(lnv[:], mv[:, :, 1], AF.Ln,
                                 bias=eps_sb[:], scale=1.0)
            r_t = stat.tile([128, GRAN], F32, tag="r", name=f"r_{bi}")
            nc.scalar.activation(r_t[:], lnv[:], AF.Exp, scale=-0.5)
            # exp bias ln(r) = -0.5*lnv folds the rstd scale into es itself
            nb = stat.tile([128, GRAN], F32, tag="nb", name=f"nb_{bi}")
            nc.vector.tensor_scalar_mul(nb[:], lnv[:], -0.5)
            # overwrite the var slot with 1/r: mv becomes [mu, 1/r] -- the
            # two trailing U columns (c1 = sum es' mu, l = sum es'/r = sum es)
            nc.scalar.activation(mv[:, :, 1], lnv[:], AF.Exp, scale=0.5)
            stage_state[bi] = (x_t, mv, r_t, nb)

        def stage_b(bi):
            """scores + exp + U accumulation for block bi (U trails scores
            by one sub-chunk so PE never waits on the exp ACT latency; the
            last sub-chunk's U spills into the next block, except at batch
            boundaries where the U banks must close for the epilogue)."""
            x_t, mv, r_t, nb = stage_state.pop(bi)
            stage_state[("u", bi)] = (x_t, mv)
            for k in range(GRAN):
                xT = work.tile([128, DIM], F16, tag="xT", name=f"xT_{bi}_{k}")
                for half in range(2):
                    tp = pt.tile([128, 384], F16, tag="tp",
                                 name=f"tp_{bi}_{k}_{half}")
                    for t in range(3):
                        et = half * 3 + t
                        nc.tensor.transpose(
                            tp[:, t * 128:(t + 1) * 128],
                            x_t[:, k, et * 128:(et + 1) * 128],
                            id_sb[:],
                        )
                    dst = xT[:, half * 384:(half + 1) * 384]
                    if half == 0:
                        nc.scalar.copy(dst, tp[:])
                    else:
                        nc.vector.tensor_copy(dst, tp[:])
                sc = pt.tile([128, J], F32, tag="tp", name=f"sc_{bi}_{k}")
                for et in range(ET):
                    nc.tensor.matmul(
                        sc[:],
                        xT[:, et * 128:(et + 1) * 128],
                        ac_sb[:, et, :],
                        start=(et == 0), stop=(et == ET - 1),
                    )
                es = work.tile([128, J], F16, tag="es", name=f"es_{bi}_{k}")
                nc.scalar.activation(es[:], sc[:], AF.Exp,
                                     bias=nb[:, k:k + 1], scale=r_t[:, k:k + 1])
                stage_state[("v", bi, k)] = es
                # U for the PREVIOUS block, interleaved between this block's
                # score matmuls: every es the PE consumes is a full block old,
                # so PE never waits on the exp ACT latency.
                if ("u", bi - 1) in stage_state:
                    u_acc(bi - 1, k)
            if ("u", bi - 1) in stage_state:
                stage_state.pop(("u", bi - 1))
            if (bi + 1) % NB_B == 0:   # last block of a batch: close U now
                for k in range(GRAN):
                    u_acc(bi, k)
                stage_state.pop(("u", bi))

        def u_acc(bi, k):
            """U accumulation for sub-chunk k of block bi."""
            gi = bi * GRAN + k
            b, c = divmod(gi, N_CHUNKS)
            x4, mv4 = stage_state[("u", bi)]
            es = stage_state.pop(("v", bi, k))
            if c == 0:
                u_tiles[b] = (
                    [pu.tile([128, 512], F32, tag=f"u{jt}", name=f"u{jt}_{b}")
                     for jt in range(JT)],
                    pu.tile([128, 512], F32, tag="uhiA", name=f"uhiA_{b}"),
                    pu.tile([128, 512], F32, tag="uhiB", name=f"uhiB_{b}"),
                )
            ulo, uhiA, uhiB = u_tiles[b]
            # start=True clears has_written for a whole PSUM bank, so in each
            # shared bank only the first-emitted matmul of chunk 0 carries
            # start=True; later first-writes land as overwrites on cleared
            # bits (start=False).
            last = (c == N_CHUNKS - 1)
            for jt in range(JT):
                nc.tensor.matmul(
                    ulo[jt][:],
                    es[:, jt * 128:(jt + 1) * 128], x4[:, k, 0:512],
                    start=(c == 0), stop=last, skip_group_check=True,
                )
            for jt in range(JT):
                dst = (uhiA[:, jt * 256:(jt + 1) * 256] if jt < 2
                       else uhiB[:, 0:256])
                nc.tensor.matmul(
                    dst,
                    es[:, jt * 128:(jt + 1) * 128], x4[:, k, 512:768],
                    start=(c == 0 and jt != 1), stop=last,
                    skip_group_check=True,
                )
            for jt in range(JT):
                nc.tensor.matmul(
                    uhiB[:, 256 + 2 * jt:258 + 2 * jt],
                    es[:, jt * 128:(jt + 1) * 128], mv4[:, k, :],
                    start=False, stop=last, skip_group_check=True,
                )

        def ep1(b):
            """pooled = (U - c1)/l evacuation (DVE/ACT only, frees U banks)."""
            ulo, uhiA, uhiB = u_tiles[b]
            p2 = epi.tile([128, JT, DIM], F16, tag="p2", name=f"p2_{b}")
            for jt in range(JT):
                rl = stat.tile([128, 1], F32, tag="rl", name=f"rl_{b}_{jt}")
                nc.vector.reciprocal(rl[:], uhiB[:, 257 + 2 * jt:258 + 2 * jt])
                cc = stat.tile([128, 1], F32, tag="cc", name=f"cc_{b}_{jt}")
                nc.scalar.copy(cc[:], uhiB[:, 256 + 2 * jt:257 + 2 * jt])
                if jt == 0:
                    # ACT path: Identity(rl*U + (-rl*c1)) == rl*(U - c1)
                    nb = stat.tile([128, 1], F32, tag="nbe", name=f"nbe_{b}")
                    nc.vector.tensor_scalar(
                        out=nb[:], in0=cc[:], scalar1=-1.0, scalar2=rl[:],
                        op0=ALU.mult, op1=ALU.mult,
                    )
                    nc.scalar.activation(
                        p2[:, jt, 0:512], ulo[jt][:],
                        AF.Identity, bias=nb[:], scale=rl[:],
                    )
                    nc.scalar.activation(
                        p2[:, jt, 512:768], uhiA[:, jt * 256:(jt + 1) * 256],
                        AF.Identity, bias=nb[:], scale=rl[:],
                    )
                    continue
                nc.vector.tensor_scalar(
                    out=p2[:, jt, 0:512], in0=ulo[jt][:],
                    scalar1=cc[:], scalar2=rl[:],
                    op0=ALU.subtract, op1=ALU.mult,
                )
                nc.vector.tensor_scalar(
                    out=p2[:, jt, 512:768],
                    in0=(uhiA[:, jt * 256:(jt + 1) * 256] if jt < 2
                         else uhiB[:, 0:256]),
                    scalar1=cc[:], scalar2=rl[:],
                    op0=ALU.subtract, op1=ALU.mult,
                )
            ep_state[b] = p2

        def ep2(b):
            """XBAR-transpose pooled -> p2T[e_local, et, j] (no PE work)."""
            p2 = ep_state.pop(b)
            p2T = epi.tile([128, ET, J], F16, tag="p2T", name=f"p2T_{b}")
            for jt in range(JT):
                nc.sync.dma_start_transpose(
                    p2T[:, :, jt * 128:(jt + 1) * 128], p2[:, jt, :]
                )
            ep_state[b] = p2T

        def ep3(b):
            """ctx = pooled_h @ Wv'_h (pre-transposed), then out = ctx @ Wo."""
            p2T = ep_state.pop(b)
            ctxT = epi.tile([128, ET, NQ], F16, tag="ctxT", name=f"ctxT_{b}")
            for h in range(H):
                cp = pt.tile([64, NQ], F32, tag="tp", name=f"cp_{b}_h{h}")
                for et in range(ET):
                    nc.tensor.matmul(
                        cp[:],
                        wv_sb[:, et, h * 64:(h + 1) * 64],
                        p2T[:, et, h * NQ:(h + 1) * NQ],
                        start=(et == 0), stop=(et == ET - 1),
                    )
                h2 = h % 2
                dst = ctxT[h2 * 64:(h2 + 1) * 64, h // 2, :]
                if h % 2 == 0:
                    nc.scalar.copy(dst, cp[:])
                else:
                    nc.vector.tensor_copy(dst, cp[:])

            # out^T: ocT[d_local, dt, n] = sum_f Wo[f, d] ctx[f, n] -- 32-col
            # moving tiles, 6x fewer PE rows than the straight orientation;
            # the host un-transposes.
            oc = epi.tile([128, ET, NQ], F32, tag="oc", name=f"oc_{b}")
            for dt in range(ET):
                po = pt.tile([128, NQ], F32, tag="tp", name=f"po_{b}_{dt}")
                for g2 in range(ET):
                    nc.tensor.matmul(
                        po[:],
                        wo_sb[:, g2, dt * 128:(dt + 1) * 128],
                        ctxT[:, g2, :],
                        start=(g2 == 0), stop=(g2 == ET - 1),
                    )
                if dt % 2 == 0:
                    nc.scalar.copy(oc[:, dt, :], po[:])
                else:
                    nc.vector.tensor_copy(oc[:, dt, :], po[:])
            nc.sync.dma_start(y_d[b], oc[:])

        NB_B = NB // B_LOC             # blocks per batch
        for bi in range(NB + 4):
            if bi < NB:
                stage_a(bi)
            if 1 <= bi < NB + 1:
                stage_st(bi - 1)
            if 2 <= bi < NB + 2:
                stage_b(bi - 2)
            # epilogue pieces trail each batch's last stage_b by 0/1/2
            # iterations so their serial chains hide behind the next batch's
            # chunk work.
            for b in range(B_LOC):
                fin = (b + 1) * NB_B + 1   # bi at which stage_b(b, last)
                if bi == fin:
                    ep1(b)
                elif bi == fin + 1:
                    ep2(b)
                elif bi == fin + 2:
                    ep3(b)

    nc.compile()
    return nc


_NC_CACHE = None


def _get_program():
    global _NC_CACHE
    if _NC_CACHE is None:
        _NC_CACHE = _build_program()
    return _NC_CACHE


def _fold_weights(queries, Wq, Wkv, Wo, gamma, beta):
    """Host-side algebraic folding of the small weights (all fp32 numpy)."""
    q = queries.astype(np.float64) @ Wq.astype(np.float64)       # [32, 768]
    qh = q.reshape(NQ, H, DH)
    Wk = Wkv[:, :INNER].astype(np.float64)
    Wv = Wkv[:, INNER:].astype(np.float64)
    Wk_h = Wk.reshape(DIM, H, DH)
    # q~[j=(h,n), e] with j head-major
    qt = np.einsum("nhd,ehd->hne", qh, Wk_h, optimize=True).reshape(J, DIM)
    A = (gamma.astype(np.float64)[:, None] * qt.T) / (DH ** 0.5)  # [768, 384]
    Ac = A - A.mean(axis=0, keepdims=True)
    Wvp = gamma.astype(np.float64)[:, None] * Wv                  # [768, 768]
    bvwo = (beta.astype(np.float64) @ Wv) @ Wo.astype(np.float64)  # [768]

    def tile6(m):  # [768, F] -> [128, 6, F] e-tile-major layout
        return np.ascontiguousarray(
            m.reshape(ET, 128, -1).transpose(1, 0, 2)
        ).astype(np.float16)

    return (
        tile6(Ac),
        tile6(Wvp),
        tile6(Wo.astype(np.float64)),
        bvwo.astype(np.float32),
    )


def kernel(encoder_outputs, queries, Wq, Wkv, Wo, ln_gamma, ln_beta):
    x = np.ascontiguousarray(
        np.asarray(encoder_outputs, dtype=np.float32).astype(np.float16)
    )
    queries = np.asarray(queries, dtype=np.float32)
    Wq = np.asarray(Wq, dtype=np.float32)
    Wkv = np.asarray(Wkv, dtype=np.float32)
    Wo_np = np.asarray(Wo, dtype=np.float32)
    gamma = np.asarray(ln_gamma, dtype=np.float32)
    beta = np.asarray(ln_beta, dtype=np.float32)

    ac_t, wv_t, wo_t, bvwo = _fold_weights(queries, Wq, Wkv, Wo_np, gamma, beta)

    nc = _get_program()
    in_maps = [
        {
            "x": x[c * B_LOC:(c + 1) * B_LOC],
            "ac": ac_t,
            "wv": wv_t,
            "wo": wo_t,
            "ident": np.eye(128, dtype=np.float16),
        }
        for c in range(N_CORES)
    ]
    res = run_bass_kernel_spmd(nc, in_maps, list(range(N_CORES)))
    y = np.concatenate([res.results[c]["y"] for c in range(N_CORES)], axis=0)
    y = y.reshape(B, 128, ET, NQ).transpose(0, 3, 2, 1).reshape(B, NQ, DIM)
    return np.ascontiguousarray(y + bvwo[None, None, :]).astype(np.float32)


# revision 41
# speedup vs baseline: 2.1308x; 1.0224x over previous
"""AttentionPooler Trainium2 kernel.

8-core data-parallel over batch (4 batches/core). Single pass over the large
encoder_outputs tensor (converted to fp16 on the host — halves HBM traffic;
all on-chip matmuls run fp16 at 1 cycle/row) with the small weights
algebraically folded on the host:

  scores[s,j] = x[s,:] @ Ac            Ac = column-centered gamma*q~^T/8
                                       (column-centering applies the
                                        LayerNorm mean subtraction exactly)
  es'[s,j] = exp(r_s*scores + ln r_s)  = r_s * exp(r_s*scores)
                                       (rstd folded into the exp bias, so
                                        the U matmul consumes RAW x — no
                                        768-wide x*r multiply on DVE)
  U[j,:]   = sum_s es'[s,j] * [x[s,:], mu_s, 1/r_s]   (PSUM accumulated)
  pooled   = (U[:, :768] - c1) / l     c1 = sum es' mu, l = sum es' / r = sum es
  ctx_h    = pooled_h @ (gamma*Wv)_h   per-head [32,768]@[768,64]
  out      = ctx @ Wo + beta@Wv@Wo

All PE transposes are replaced by XBAR DMA transposes (fp16-only HW path):
x^T comes straight from DRAM, pooled^T from SBUF.
"""
import numpy as np

import concourse.bass as bass
import concourse.bacc as bacc
import concourse.tile as tile
from concourse import mybir
from concourse.bass_utils import run_bass_kernel_spmd

# ---- problem constants (hardcoded per harness contract) ----
B, S, DIM = 32, 4096, 768
H, NQ, DH = 12, 32, 64
INNER = H * DH          # 768
J = H * NQ              # 384
N_CORES = 8
B_LOC = B // N_CORES    # 4
CHUNK = 128
N_CHUNKS = S // CHUNK   # 32
ET = DIM // 128         # 6 e-tiles of the model dim
JT = J // 128           # 3 j-tiles
EPS = 1e-5

F32 = mybir.dt.float32
F16 = mybir.dt.float16
AF = mybir.ActivationFunctionType
ALU = mybir.AluOpType


def _steer_act_tables(arch: str):
    """Make the act-table-load pass serve Exp from the set that also holds
    Ln, so a kernel alternating Ln/Exp loads tables exactly once."""
    from concourse.hw_specs import get_activation_tables

    tables = get_activation_tables(arch)  # functools.cache -> shared dict
    keep = "natural_log_exp_and_others"
    if keep in tables:
        for name, funcs in tables.items():
            if name != keep:
                funcs.discard(AF.Exp)


def _build_program():
    nc = bacc.Bacc(
        "TRN2", target_bir_lowering=False, debug=False, num_devices=N_CORES
    )
    _steer_act_tables(nc.m.arch)
    x_d = nc.dram_tensor("x", [B_LOC, S, DIM], F16, kind="ExternalInput")
    ac_d = nc.dram_tensor("ac", [128, ET, J], F16, kind="ExternalInput")
    wv_d = nc.dram_tensor("wv", [128, ET, INNER], F16, kind="ExternalInput")
    wo_d = nc.dram_tensor("wo", [128, ET, DIM], F16, kind="ExternalInput")
    id_d = nc.dram_tensor("ident", [128, 128], F16, kind="ExternalInput")
    y_d = nc.dram_tensor("y", [B_LOC, ET, 128, NQ], F32, kind="ExternalOutput")

    with tile.TileContext(nc) as tc, \
         tc.tile_pool(name="const", bufs=1) as const, \
         tc.tile_pool(name="xin", bufs=4) as xin, \
         tc.tile_pool(name="work", bufs=5) as work, \
         tc.tile_pool(name="stat", bufs=8) as stat, \
         tc.tile_pool(name="epi", bufs=2) as epi, \
         tc.tile_pool(name="pu", bufs=1, space="PSUM") as pu, \
         tc.tile_pool(name="pt", bufs=3, space="PSUM") as pt:

        ac_sb = const.tile([128, ET, J], F16, tag="ac")
        # wv/wo are first needed ~70us in (first epilogue); issue their DMAs
        # a few chunks into batch 0 so the first x chunks aren't queued
        # behind the weights.
        wv_sb = const.tile([128, ET, INNER], F16, tag="wv")
        wo_sb = const.tile([128, ET, DIM], F16, tag="wo")
        eps_sb = const.tile([128, 1], F32, tag="eps")
        nc.vector.memset(eps_sb[:], EPS)
        # ac first on the SP ring: the first scores need it ~6us in
        nc.sync.dma_start(ac_sb[:], ac_d[:])
        id_sb = const.tile([128, 128], F16, tag="ident")
        nc.sync.dma_start(id_sb[:], id_d[:])

        # ac first on the SP ring: the first scores need it ~6us in
        nc.sync.dma_start(ac_sb[:], ac_d[:])

        TOT = B_LOC * N_CHUNKS
        u_tiles = {}
        stage_state = {}
        ep_state = {}

        GRAN = 4                       # chunks per DMA block (512 seq rows)
        NB = TOT // GRAN               # 32 blocks/core, 8 per batch

        def stage_a(bi):
            """Block DMA issue: one straight load (SP ring) + one XBAR
            transpose (ACT ring) covering GRAN chunks = 512 seq rows."""
            b, c0 = divmod(bi * GRAN, N_CHUNKS)
            src = x_d[b, c0 * 128:(c0 + GRAN) * 128, :]
            x_t = xin.tile([128, GRAN, DIM], F16, tag="x", name=f"x_{bi}")
            if bi == 0:
                for k in range(GRAN):
                    nc.sync.dma_start(
                        x_t[:, k, :], x_d[b, k * 128:(k + 1) * 128, :]
                    )
            else:
                nc.sync.dma_start(
                    x_t[:], src.rearrange("(sub p) e -> p sub e", p=128)
                )
            if bi == 2:
                nc.sync.dma_start(wv_sb[:], wv_d[:])
            if bi == 3:
                nc.sync.dma_start(wo_sb[:], wo_d[:])
            stage_state[("d", bi)] = x_t

        def stage_st(bi):
            """LayerNorm row stats for all GRAN sub-chunks of block bi."""
            x_t = stage_state.pop(("d", bi))
            st = stat.tile([128, 2 * GRAN, 6], F32, tag="st", name=f"st_{bi}")
            xg = x_t[:].rearrange("p sub (n f) -> p (sub n) f", f=384)
            for g in range(2 * GRAN):
                nc.vector.bn_stats(st[:, g, :], xg[:, g, :])
            mv = stat.tile([128, GRAN, 2], F16, tag="mv", name=f"mv_{bi}")
            for k in range(GRAN):
                nc.vector.bn_aggr(mv[:, k, :], st[:, 2 * k:2 * k + 2, :])
            # r = (var+eps)^-1/2 = exp(-0.5*ln(var+eps)); Ln+Exp share an ACT
            # table set (Rsqrt activation is banned for accuracy).
            lnv = stat.tile([128, GRAN], F32, tag="lnv", name=f"lnv_{bi}")
            nc.scalar.activation(lnv[:], mv[:, :, 1], AF.Ln,
                                 bias=eps_sb[:], scale=1.0)
            r_t = stat.tile([128, GRAN], F32, tag="r", name=f"r_{bi}")
            nc.scalar.activation(r_t[:], lnv[:], AF.Exp, scale=-0.5)
            # exp bias ln(r) = -0.5*lnv folds the rstd scale into es itself
            nb = stat.tile([128, GRAN], F32, tag="nb", name=f"nb_{bi}")
            nc.vector.tensor_scalar_mul(nb[:], lnv[:], -0.5)
            # overwrite the var slot with 1/r: mv becomes [mu, 1/r] -- the
            # two trailing U columns (c1 = sum es' mu, l = sum es'/r = sum es)
            nc.scalar.activation(mv[:, :, 1], lnv[:], AF.Exp, scale=0.5)
            stage_state[bi] = (x_t, mv, r_t, nb)

        def stage_b(bi):
            """scores + exp + U accumulation for block bi (U trails scores
            by one sub-chunk so PE never waits on the exp ACT latency; the
            last sub-chunk's U spills into the next block, except at batch
            boundaries where the U banks must close for the epilogue)."""
            x_t, mv, r_t, nb = stage_state.pop(bi)
            stage_state[("u", bi)] = (x_t, mv)
            for k in range(GRAN):
                xT = work.tile([128, DIM], F16, tag="xT", name=f"xT_{bi}_{k}")
                for half in range(2):
                    tp = pt.tile([128, 384], F16, tag="tp",
                                 name=f"tp_{bi}_{k}_{half}")
                    for t in range(3):
                        et = half * 3 + t
                        nc.tensor.transpose(
                            tp[:, t * 128:(t + 1) * 128],
                            x_t[:, k, et * 128:(et + 1) * 128],
                            id_sb[:],
                        )
                    dst = xT[:, half * 384:(half + 1) * 384]
                    if half == 0:
                        nc.scalar.copy(dst, tp[:])
                    else:
                        nc.vector.tensor_copy(dst, tp[:])
                sc = pt.tile([128, J], F32, tag="tp", name=f"sc_{bi}_{k}")
                for et in range(ET):
                    nc.tensor.matmul(
                        sc[:],
                        xT[:, et * 128:(et + 1) * 128],
                        ac_sb[:, et, :],
                        start=(et == 0), stop=(et == ET - 1),
                    )
                es = work.tile([128, J], F16, tag="es", name=f"es_{bi}_{k}")
                nc.scalar.activation(es[:], sc[:], AF.Exp,
                                     bias=nb[:, k:k + 1], scale=r_t[:, k:k + 1])
                stage_state[("v", bi, k)] = es
                # U for the PREVIOUS block, interleaved between this block's
                # score matmuls: every es the PE consumes is a full block old,
                # so PE never waits on the exp ACT latency.
                if ("u", bi - 1) in stage_state:
                    u_acc(bi - 1, k)
            if ("u", bi - 1) in stage_state:
                stage_state.pop(("u", bi - 1))
            if (bi + 1) % NB_B == 0:   # last block of a batch: close U now
                for k in range(GRAN):
                    u_acc(bi, k)
                stage_state.pop(("u", bi))

        def u_acc(bi, k):
            """U accumulation for sub-chunk k of block bi."""
            gi = bi * GRAN + k
            b, c = divmod(gi, N_CHUNKS)
            x4, mv4 = stage_state[("u", bi)]
            es = stage_state.pop(("v", bi, k))
            if c == 0:
                u_tiles[b] = (
                    [pu.tile([128, 512], F32, tag=f"u{jt}", name=f"u{jt}_{b}")
                     for jt in range(JT)],
                    pu.tile([128, 512], F32, tag="uhiA", name=f"uhiA_{b}"),
                    pu.tile([128, 512], F32, tag="uhiB", name=f"uhiB_{b}"),
                )
            ulo, uhiA, uhiB = u_tiles[b]
            # start=True clears has_written for a whole PSUM bank, so in each
            # shared bank only the first-emitted matmul of chunk 0 carries
            # start=True; later first-writes land as overwrites on cleared
            # bits (start=False).
            last = (c == N_CHUNKS - 1)
            for jt in range(JT):
                nc.tensor.matmul(
                    ulo[jt][:],
                    es[:, jt * 128:(jt + 1) * 128], x4[:, k, 0:512],
                    start=(c == 0), stop=last, skip_group_check=True,
                )
            for jt in range(JT):
                dst = (uhiA[:, jt * 256:(jt + 1) * 256] if jt < 2
                       else uhiB[:, 0:256])
                nc.tensor.matmul(
                    dst,
                    es[:, jt * 128:(jt + 1) * 128], x4[:, k, 512:768],
                    start=(c == 0 and jt != 1), stop=last,
                    skip_group_check=True,
                )
            for jt in range(JT):
                nc.tensor.matmul(
                    uhiB[:, 256 + 2 * jt:258 + 2 * jt],
                    es[:, jt * 128:(jt + 1) * 128], mv4[:, k, :],
                    start=False, stop=last, skip_group_check=True,
                )

        def ep1(b):
            """pooled = (U - c1)/l evacuation (DVE/ACT only, frees U banks)."""
            ulo, uhiA, uhiB = u_tiles[b]
            p2 = epi.tile([128, JT, DIM], F16, tag="p2", name=f"p2_{b}")
            for jt in range(JT):
                rl = stat.tile([128, 1], F32, tag="rl", name=f"rl_{b}_{jt}")
                nc.vector.reciprocal(rl[:], uhiB[:, 257 + 2 * jt:258 + 2 * jt])
                cc = stat.tile([128, 1], F32, tag="cc", name=f"cc_{b}_{jt}")
                nc.scalar.copy(cc[:], uhiB[:, 256 + 2 * jt:257 + 2 * jt])
                if jt == 0:
                    # ACT path: Identity(rl*U + (-rl*c1)) == rl*(U - c1)
                    nb = stat.tile([128, 1], F32, tag="nbe", name=f"nbe_{b}")
                    nc.vector.tensor_scalar(
                        out=nb[:], in0=cc[:], scalar1=-1.0, scalar2=rl[:],
                        op0=ALU.mult, op1=ALU.mult,
                    )
                    nc.scalar.activation(
                        p2[:, jt, 0:512], ulo[jt][:],
                        AF.Identity, bias=nb[:], scale=rl[:],
                    )
                    nc.scalar.activation(
                        p2[:, jt, 512:768], uhiA[:, jt * 256:(jt + 1) * 256],
                        AF.Identity, bias=nb[:], scale=rl[:],
                    )
                    continue
                nc.vector.tensor_scalar(
                    out=p2[:, jt, 0:512], in0=ulo[jt][:],
                    scalar1=cc[:], scalar2=rl[:],
                    op0=ALU.subtract, op1=ALU.mult,
                )
                nc.vector.tensor_scalar(
                    out=p2[:, jt, 512:768],
                    in0=(uhiA[:, jt * 256:(jt + 1) * 256] if jt < 2
                         else uhiB[:, 0:256]),
                    scalar1=cc[:], scalar2=rl[:],
                    op0=ALU.subtract, op1=ALU.mult,
                )
            ep_state[b] = p2

        def ep2(b):
            """XBAR-transpose pooled -> p2T[e_local, et, j] (no PE work)."""
            p2 = ep_state.pop(b)
            p2T = epi.tile([128, ET, J], F16, tag="p2T", name=f"p2T_{b}")
            for jt in range(JT):
                nc.sync.dma_start_transpose(
                    p2T[:, :, jt * 128:(jt + 1) * 128], p2[:, jt, :]
                )
            ep_state[b] = p2T

        def ep3(b):
            """ctx = pooled_h @ Wv'_h (pre-transposed), then out = ctx @ Wo."""
            p2T = ep_state.pop(b)
            ctxT = epi.tile([128, ET, NQ], F16, tag="ctxT", name=f"ctxT_{b}")
            for h in range(H):
                cp = pt.tile([64, NQ], F32, tag="tp", name=f"cp_{b}_h{h}")
                for et in range(ET):
                    nc.tensor.matmul(
                        cp[:],
                        wv_sb[:, et, h * 64:(h + 1) * 64],
                        p2T[:, et, h * NQ:(h + 1) * NQ],
                        start=(et == 0), stop=(et == ET - 1),
                    )
                h2 = h % 2
                dst = ctxT[h2 * 64:(h2 + 1) * 64, h // 2, :]
                if h % 2 == 0:
                    nc.scalar.copy(dst, cp[:])
                else:
                    nc.vector.tensor_copy(dst, cp[:])

            # out^T: ocT[d_local, dt, n] = sum_f Wo[f, d] ctx[f, n] -- 32-col
            # moving tiles, 6x fewer PE rows than the straight orientation;
            # the host un-transposes.
            oc = epi.tile([128, ET, NQ], F32, tag="oc", name=f"oc_{b}")
            for dt in range(ET):
                po = pt.tile([128, NQ], F32, tag="tp", name=f"po_{b}_{dt}")
                for g2 in range(ET):
                    nc.tensor.matmul(
                        po[:],
                        wo_sb[:, g2, dt * 128:(dt + 1) * 128],
                        ctxT[:, g2, :],
                        start=(g2 == 0), stop=(g2 == ET - 1),
                    )
                if dt % 2 == 0:
                    nc.scalar.copy(oc[:, dt, :], po[:])
                else:
                    nc.vector.tensor_copy(oc[:, dt, :], po[:])
            nc.sync.dma_start(y_d[b], oc[:])

        NB_B = NB // B_LOC             # blocks per batch
        for bi in range(NB + 4):
            if bi < NB:
                stage_a(bi)
            if 1 <= bi < NB + 1:
                stage_st(bi - 1)
            if 2 <= bi < NB + 2:
                stage_b(bi - 2)
            # epilogue pieces trail each batch's last stage_b by 0/1/2
            # iterations so their serial chains hide behind the next batch's
            # chunk work.
            for b in range(B_LOC):
                fin = (b + 1) * NB_B + 1   # bi at which stage_b(b, last)
                if bi == fin:
                    ep1(b)
                elif bi == fin + 1:
                    ep2(b)
                elif bi == fin + 2:
                    ep3(b)

    nc.compile()
    return nc


_NC_CACHE = None


def _get_program():
    global _NC_CACHE
    if _NC_CACHE is None:
        _NC_CACHE = _build_program()
    return _NC_CACHE


def _fold_weights(queries, Wq, Wkv, Wo, gamma, beta):
    """Host-side algebraic folding of the small weights (all fp32 numpy)."""
    q = queries.astype(np.float64) @ Wq.astype(np.float64)       # [32, 768]
    qh = q.reshape(NQ, H, DH)
    Wk = Wkv[:, :INNER].astype(np.float64)
    Wv = Wkv[:, INNER:].astype(np.float64)
    Wk_h = Wk.reshape(DIM, H, DH)
    # q~[j=(h,n), e] with j head-major
    qt = np.einsum("nhd,ehd->hne", qh, Wk_h, optimize=True).reshape(J, DIM)
    A = (gamma.astype(np.float64)[:, None] * qt.T) / (DH ** 0.5)  # [768, 384]
    Ac = A - A.mean(axis=0, keepdims=True)
    Wvp = gamma.astype(np.float64)[:, None] * Wv                  # [768, 768]
    bvwo = (beta.astype(np.float64) @ Wv) @ Wo.astype(np.float64)  # [768]

    def tile6(m):  # [768, F] -> [128, 6, F] e-tile-major layout
        return np.ascontiguousarray(
            m.reshape(ET, 128, -1).transpose(1, 0, 2)
        ).astype(np.float16)

    return (
        tile6(Ac),
        tile6(Wvp),
        tile6(Wo.astype(np.float64)),
        bvwo.astype(np.float32),
    )


def kernel(encoder_outputs, queries, Wq, Wkv, Wo, ln_gamma, ln_beta):
    x = np.ascontiguousarray(
        np.asarray(encoder_outputs, dtype=np.float32).astype(np.float16)
    )
    queries = np.asarray(queries, dtype=np.float32)
    Wq = np.asarray(Wq, dtype=np.float32)
    Wkv = np.asarray(Wkv, dtype=np.float32)
    Wo_np = np.asarray(Wo, dtype=np.float32)
    gamma = np.asarray(ln_gamma, dtype=np.float32)
    beta = np.asarray(ln_beta, dtype=np.float32)

    ac_t, wv_t, wo_t, bvwo = _fold_weights(queries, Wq, Wkv, Wo_np, gamma, beta)

    nc = _get_program()
    in_maps = [
        {
            "x": x[c * B_LOC:(c + 1) * B_LOC],
            "ac": ac_t,
            "wv": wv_t,
            "wo": wo_t,
            "ident": np.eye(128, dtype=np.float16),
        }
        for c in range(N_CORES)
    ]
    res = run_bass_kernel_spmd(nc, in_maps, list(range(N_CORES)))
    y = np.concatenate([res.results[c]["y"] for c in range(N_CORES)], axis=0)
    y = y.reshape(B, 128, ET, NQ).transpose(0, 3, 2, 1).reshape(B, NQ, DIM)
    return np.ascontiguousarray(y + bvwo[None, None, :]).astype(np.float32)


# revision 59
# speedup vs baseline: 2.2590x; 1.0602x over previous
"""AttentionPooler Trainium2 kernel.

8-core data-parallel over batch (4 batches/core). Single pass over the large
encoder_outputs tensor (converted to fp16 on the host — halves HBM traffic;
all on-chip matmuls run fp16 at 1 cycle/row) with the small weights
algebraically folded on the host:

  scores[s,j] = x[s,:] @ Ac            Ac = column-centered gamma*q~^T/8
                                       (column-centering applies the
                                        LayerNorm mean subtraction exactly)
  es'[s,j] = exp(r_s*scores + ln r_s)  = r_s * exp(r_s*scores)
                                       (rstd folded into the exp bias, so
                                        the U matmul consumes RAW x — no
                                        768-wide x*r multiply on DVE)
  U[j,:]   = sum_s es'[s,j] * [x[s,:], mu_s, 1/r_s]   (PSUM accumulated)
  pooled   = (U[:, :768] - c1) / l     c1 = sum es' mu, l = sum es' / r = sum es
  ctx_h    = pooled_h @ (gamma*Wv)_h   per-head [32,768]@[768,64]
  out      = ctx @ Wo + beta@Wv@Wo

All PE transposes are replaced by XBAR DMA transposes (fp16-only HW path):
x^T comes straight from DRAM, pooled^T from SBUF.
"""
import numpy as np

import concourse.bass as bass
import concourse.bacc as bacc
import concourse.tile as tile
from concourse import mybir
from concourse.bass_utils import run_bass_kernel_spmd

# ---- problem constants (hardcoded per harness contract) ----
B, S, DIM = 32, 4096, 768
H, NQ, DH = 12, 32, 64
INNER = H * DH          # 768
J = H * NQ              # 384
N_CORES = 8
B_LOC = B // N_CORES    # 4
CHUNK = 128
N_CHUNKS = S // CHUNK   # 32
ET = DIM // 128         # 6 e-tiles of the model dim
JT = J // 128           # 3 j-tiles
EPS = 1e-5

F32 = mybir.dt.float32
F16 = mybir.dt.float16
AF = mybir.ActivationFunctionType
ALU = mybir.AluOpType


def _steer_act_tables(arch: str):
    """Make the act-table-load pass serve Exp from the set that also holds
    Ln, so a kernel alternating Ln/Exp loads tables exactly once."""
    from concourse.hw_specs import get_activation_tables

    tables = get_activation_tables(arch)  # functools.cache -> shared dict
    keep = "natural_log_exp_and_others"
    if keep in tables:
        for name, funcs in tables.items():
            if name != keep:
                funcs.discard(AF.Exp)


def _build_program():
    nc = bacc.Bacc(
        "TRN2", target_bir_lowering=False, debug=False, num_devices=N_CORES
    )
    _steer_act_tables(nc.m.arch)
    x_d = nc.dram_tensor("x", [B_LOC, S, DIM], F16, kind="ExternalInput")
    ac_d = nc.dram_tensor("ac", [128, ET, J], F16, kind="ExternalInput")
    wv_d = nc.dram_tensor("wv", [128, ET, INNER], F16, kind="ExternalInput")
    wo_d = nc.dram_tensor("wo", [128, ET, DIM], F16, kind="ExternalInput")
    id_d = nc.dram_tensor("ident", [128, 128], F16, kind="ExternalInput")
    y_d = nc.dram_tensor("y", [B_LOC, ET, 128, NQ], F32, kind="ExternalOutput")

    with tile.TileContext(nc) as tc, \
         tc.tile_pool(name="const", bufs=1) as const, \
         tc.tile_pool(name="xin", bufs=5) as xin, \
         tc.tile_pool(name="work", bufs=12) as work, \
         tc.tile_pool(name="stat", bufs=8) as stat, \
         tc.tile_pool(name="epi", bufs=2) as epi, \
         tc.tile_pool(name="pu", bufs=1, space="PSUM") as pu, \
         tc.tile_pool(name="pt", bufs=3, space="PSUM") as pt:

        ac_sb = const.tile([128, ET, J], F16, tag="ac")
        # wv/wo are first needed ~70us in (first epilogue); issue their DMAs
        # a few chunks into batch 0 so the first x chunks aren't queued
        # behind the weights.
        wv_sb = const.tile([128, ET, INNER], F16, tag="wv")
        wo_sb = const.tile([128, ET, DIM], F16, tag="wo")
        eps_sb = const.tile([128, 1], F32, tag="eps")
        nc.vector.memset(eps_sb[:], EPS)
        id_sb = const.tile([128, 128], F16, tag="ident")
        nc.scalar.dma_start(id_sb[:], id_d[:])

        TOT = B_LOC * N_CHUNKS
        u_tiles = {}
        stage_state = {}
        ep_state = {}

        GRAN = 4                       # chunks per DMA block (512 seq rows)
        NB = TOT // GRAN               # 32 blocks/core, 8 per batch

        def stage_a(bi):
            """Block DMA issue: one straight load (SP ring) + one XBAR
            transpose (ACT ring) covering GRAN chunks = 512 seq rows."""
            b, c0 = divmod(bi * GRAN, N_CHUNKS)
            src = x_d[b, c0 * 128:(c0 + GRAN) * 128, :]
            x_t = xin.tile([128, GRAN, DIM], F16, tag="x", name=f"x_{bi}")
            if bi == 0:
                for k in range(GRAN):
                    nc.sync.dma_start(
                        x_t[:, k, :], x_d[b, k * 128:(k + 1) * 128, :]
                    )
                    if k == 0:
                        # ac right behind chunk 0: scores(0,0) get it just
                        # in time without delaying the first stats chunk
                        nc.sync.dma_start(ac_sb[:], ac_d[:])

            else:
                nc.sync.dma_start(
                    x_t[:], src.rearrange("(sub p) e -> p sub e", p=128)
                )
            if bi == 2:
                nc.sync.dma_start(wv_sb[:], wv_d[:])
            if bi == 3:
                nc.sync.dma_start(wo_sb[:], wo_d[:])
            stage_state[("d", bi)] = x_t

        def stage_st(bi):
            """LayerNorm row stats for all GRAN sub-chunks of block bi."""
            x_t = stage_state.pop(("d", bi))
            st = stat.tile([128, 2 * GRAN, 6], F32, tag="st", name=f"st_{bi}")
            xg = x_t[:].rearrange("p sub (n f) -> p (sub n) f", f=384)
            for g in range(2 * GRAN):
                nc.vector.bn_stats(st[:, g, :], xg[:, g, :])
            mv = stat.tile([128, GRAN, 2], F16, tag="mv", name=f"mv_{bi}")
            for k in range(GRAN):
                nc.vector.bn_aggr(mv[:, k, :], st[:, 2 * k:2 * k + 2, :])
            # r = (var+eps)^-1/2 = exp(-0.5*ln(var+eps)); Ln+Exp share an ACT
            # table set (Rsqrt activation is banned for accuracy).
            lnv = stat.tile([128, GRAN], F32, tag="lnv", name=f"lnv_{bi}")
            nc.scalar.activation(lnv[:], mv[:, :, 1], AF.Ln,
                                 bias=eps_sb[:], scale=1.0)
            r_t = stat.tile([128, GRAN], F32, tag="r", name=f"r_{bi}")
            nc.scalar.activation(r_t[:], lnv[:], AF.Exp, scale=-0.5)
            # exp bias ln(r) = -0.5*lnv folds the rstd scale into es itself
            nb = stat.tile([128, GRAN], F32, tag="nb", name=f"nb_{bi}")
            nc.vector.tensor_scalar_mul(nb[:], lnv[:], -0.5)
            # overwrite the var slot with 1/r: mv becomes [mu, 1/r] -- the
            # two trailing U columns (c1 = sum es' mu, l = sum es'/r = sum es)
            nc.scalar.activation(mv[:, :, 1], lnv[:], AF.Exp, scale=0.5)
            stage_state[bi] = (x_t, mv, r_t, nb)

        def stage_b(bi):
            """scores + exp + U accumulation for block bi (U trails scores
            by one sub-chunk so PE never waits on the exp ACT latency; the
            last sub-chunk's U spills into the next block, except at batch
            boundaries where the U banks must close for the epilogue)."""
            x_t, mv, r_t, nb = stage_state.pop(bi)
            stage_state[("u", bi)] = (x_t, mv)
            for k in range(GRAN):
                xT = work.tile([128, DIM], F16, tag="xT", name=f"xT_{bi}_{k}")
                for half in range(2):
                    tp = pt.tile([128, 384], F16, tag="tp",
                                 name=f"tp_{bi}_{k}_{half}")
                    for t in range(3):
                        et = half * 3 + t
                        nc.tensor.transpose(
                            tp[:, t * 128:(t + 1) * 128],
                            x_t[:, k, et * 128:(et + 1) * 128],
                            id_sb[:],
                        )
                    dst = xT[:, half * 384:(half + 1) * 384]
                    if half == 0:
                        nc.scalar.copy(dst, tp[:])
                    else:
                        nc.vector.tensor_copy(dst, tp[:])
                sc = pt.tile([128, J], F32, tag="tp", name=f"sc_{bi}_{k}")
                for et in range(ET):
                    nc.tensor.matmul(
                        sc[:],
                        xT[:, et * 128:(et + 1) * 128],
                        ac_sb[:, et, :],
                        start=(et == 0), stop=(et == ET - 1),
                    )
                es = work.tile([128, J], F16, tag="es", name=f"es_{bi}_{k}")
                nc.scalar.activation(es[:], sc[:], AF.Exp,
                                     bias=nb[:, k:k + 1], scale=r_t[:, k:k + 1])
                stage_state[("v", bi, k)] = es
                # U for the PREVIOUS block, interleaved between this block's
                # score matmuls: every es the PE consumes is a full block old,
                # so PE never waits on the exp ACT latency.
                if ("u", bi - 1) in stage_state:
                    u_acc(bi - 1, k)
            if ("u", bi - 1) in stage_state:
                stage_state.pop(("u", bi - 1))
            if (bi + 1) % NB_B == 0:   # last block of a batch: close U now
                for k in range(GRAN):
                    u_acc(bi, k)
                stage_state.pop(("u", bi))

        def u_acc(bi, k):
            """U accumulation for sub-chunk k of block bi."""
            gi = bi * GRAN + k
            b, c = divmod(gi, N_CHUNKS)
            x4, mv4 = stage_state[("u", bi)]
            es = stage_state.pop(("v", bi, k))
            if c == 0:
                u_tiles[b] = (
                    [pu.tile([128, 512], F32, tag=f"u{jt}", name=f"u{jt}_{b}")
                     for jt in range(JT)],
                    pu.tile([128, 512], F32, tag="uhiA", name=f"uhiA_{b}"),
                    pu.tile([128, 512], F32, tag="uhiB", name=f"uhiB_{b}"),
                )
            ulo, uhiA, uhiB = u_tiles[b]
            # start=True clears has_written for a whole PSUM bank, so in each
            # shared bank only the first-emitted matmul of chunk 0 carries
            # start=True; later first-writes land as overwrites on cleared
            # bits (start=False).
            last = (c == N_CHUNKS - 1)
            for jt in range(JT):
                nc.tensor.matmul(
                    ulo[jt][:],
                    es[:, jt * 128:(jt + 1) * 128], x4[:, k, 0:512],
                    start=(c == 0), stop=last, skip_group_check=True,
                )
            for jt in range(JT):
                dst = (uhiA[:, jt * 256:(jt + 1) * 256] if jt < 2
                       else uhiB[:, 0:256])
                nc.tensor.matmul(
                    dst,
                    es[:, jt * 128:(jt + 1) * 128], x4[:, k, 512:768],
                    start=(c == 0 and jt != 1), stop=last,
                    skip_group_check=True,
                )
            for jt in range(JT):
                nc.tensor.matmul(
                    uhiB[:, 256 + 2 * jt:258 + 2 * jt],
                    es[:, jt * 128:(jt + 1) * 128], mv4[:, k, :],
                    start=False, stop=last, skip_group_check=True,
                )

        def ep1(b):
            """pooled = (U - c1)/l evacuation (DVE/ACT only, frees U banks)."""
            ulo, uhiA, uhiB = u_tiles[b]
            p2 = epi.tile([128, JT, DIM], F16, tag="p2", name=f"p2_{b}")
            for jt in range(JT):
                rl = stat.tile([128, 1], F32, tag="rl", name=f"rl_{b}_{jt}")
                nc.vector.reciprocal(rl[:], uhiB[:, 257 + 2 * jt:258 + 2 * jt])
                cc = stat.tile([128, 1], F32, tag="cc", name=f"cc_{b}_{jt}")
                nc.scalar.copy(cc[:], uhiB[:, 256 + 2 * jt:257 + 2 * jt])
                # all on DVE: the ACT engine is es-exp-critical exactly
                # when epilogues run (batch boundaries)
                nc.vector.tensor_scalar(
                    out=p2[:, jt, 0:512], in0=ulo[jt][:],
                    scalar1=cc[:], scalar2=rl[:],
                    op0=ALU.subtract, op1=ALU.mult,
                )
                nc.vector.tensor_scalar(
                    out=p2[:, jt, 512:768],
                    in0=(uhiA[:, jt * 256:(jt + 1) * 256] if jt < 2
                         else uhiB[:, 0:256]),
                    scalar1=cc[:], scalar2=rl[:],
                    op0=ALU.subtract, op1=ALU.mult,
                )
            ep_state[b] = p2

        def ep2(b):
            """transpose pooled -> p2T[e_local, et, j]: XBAR mid-stream (free
            DMA, hides behind chunk work), PE for the last batch (the tail
            has nothing to hide a DMA round trip behind)."""
            p2 = ep_state.pop(b)
            p2T = epi.tile([128, ET, J], F16, tag="p2T", name=f"p2T_{b}")
            if b < B_LOC - 1:
                for jt in range(JT):
                    nc.sync.dma_start_transpose(
                        p2T[:, :, jt * 128:(jt + 1) * 128], p2[:, jt, :]
                    )
            else:
                for et in range(ET):
                    tp = pt.tile([128, 384], F16, tag="tp",
                                 name=f"ep_tp_{b}_{et}")
                    for jt in range(JT):
                        nc.tensor.transpose(
                            tp[:, jt * 128:(jt + 1) * 128],
                            p2[:, jt, et * 128:(et + 1) * 128],
                            id_sb[:],
                        )
                    if et % 2 == 0:
                        nc.scalar.copy(p2T[:, et, :], tp[:])
                    else:
                        nc.vector.tensor_copy(p2T[:, et, :], tp[:])
            ep_state[b] = p2T

        def ep3(b):
            """ctx = pooled_h @ Wv'_h (pre-transposed), then out = ctx @ Wo."""
            p2T = ep_state.pop(b)
            ctxT = epi.tile([128, ET, NQ], F16, tag="ctxT", name=f"ctxT_{b}")
            for g in range(3):
                # 4 heads per PSUM bank (2 partition halves x 2 col groups):
                # one wide evacuation copy instead of four tiny ones
                cp = pt.tile([128, 2, NQ], F32, tag="tp", name=f"cp_{b}_g{g}")
                for hh in range(4):
                    h = g * 4 + hh
                    dst = cp[(hh % 2) * 64:(hh % 2) * 64 + 64, hh // 2, :]
                    for et in range(ET):
                        # start=True pends-zero the 2KB bank only on the
                        # partitions this matmul writes, so EACH partition
                        # half needs one bank-clearing first write (hh 0/1)
                        nc.tensor.matmul(
                            dst,
                            wv_sb[:, et, h * 64:(h + 1) * 64],
                            p2T[:, et, h * NQ:(h + 1) * NQ],
                            start=(et == 0 and hh <= 1), stop=(et == ET - 1),
                            skip_group_check=True,
                        )
                if g % 2 == 0:
                    nc.scalar.copy(ctxT[:, 2 * g:2 * g + 2, :], cp[:])
                else:
                    nc.vector.tensor_copy(ctxT[:, 2 * g:2 * g + 2, :], cp[:])

            # out^T: ocT[d_local, dt, n] = sum_f Wo[f, d] ctx[f, n] -- 32-col
            # moving tiles, 6x fewer PE rows than the straight orientation;
            # the host un-transposes.
            oc = epi.tile([128, ET, NQ], F32, tag="oc", name=f"oc_{b}")
            for g in range(2):
                # 3 output d-tiles per PSUM bank, one wide evacuation copy
                po = pt.tile([128, 3, NQ], F32, tag="tp", name=f"po_{b}_{g}")
                for dd in range(3):
                    dt = g * 3 + dd
                    for g2 in range(ET):
                        nc.tensor.matmul(
                            po[:, dd, :],
                            wo_sb[:, g2, dt * 128:(dt + 1) * 128],
                            ctxT[:, g2, :],
                            start=(g2 == 0 and dd == 0), stop=(g2 == ET - 1),
                            skip_group_check=True,
                        )
                if g == 0:
                    nc.scalar.copy(oc[:, 0:3, :], po[:])
                else:
                    nc.vector.tensor_copy(oc[:, 3:6, :], po[:])
            nc.sync.dma_start(y_d[b], oc[:])

        NB_B = NB // B_LOC             # blocks per batch
        for bi in range(NB + 4):
            if bi < NB:
                stage_a(bi)
            if 1 <= bi < NB + 1:
                stage_st(bi - 1)
            if 2 <= bi < NB + 2:
                stage_b(bi - 2)
            # epilogue pieces trail each batch's last stage_b by 0/1/2
            # iterations so their serial chains hide behind the next batch's
            # chunk work.
            for b in range(B_LOC):
                fin = (b + 1) * NB_B + 1   # bi at which stage_b(b, last)
                if bi == fin:
                    ep1(b)
                elif bi == fin + 1:
                    ep2(b)
                elif bi == fin + 2:
                    ep3(b)

    nc.compile()
    return nc


_NC_CACHE = None


def _get_program():
    global _NC_CACHE
    if _NC_CACHE is None:
        _NC_CACHE = _build_program()
    return _NC_CACHE


def _fold_weights(queries, Wq, Wkv, Wo, gamma, beta):
    """Host-side algebraic folding of the small weights (all fp32 numpy)."""
    q = queries.astype(np.float64) @ Wq.astype(np.float64)       # [32, 768]
    qh = q.reshape(NQ, H, DH)
    Wk = Wkv[:, :INNER].astype(np.float64)
    Wv = Wkv[:, INNER:].astype(np.float64)
    Wk_h = Wk.reshape(DIM, H, DH)
    # q~[j=(h,n), e] with j head-major
    qt = np.einsum("nhd,ehd->hne", qh, Wk_h, optimize=True).reshape(J, DIM)
    A = (gamma.astype(np.float64)[:, None] * qt.T) / (DH ** 0.5)  # [768, 384]
    Ac = A - A.mean(axis=0, keepdims=True)
    Wvp = gamma.astype(np.float64)[:, None] * Wv                  # [768, 768]
    bvwo = (beta.astype(np.float64) @ Wv) @ Wo.astype(np.float64)  # [768]

    def tile6(m):  # [768, F] -> [128, 6, F] e-tile-major layout
        return np.ascontiguousarray(
            m.reshape(ET, 128, -1).transpose(1, 0, 2)
        ).astype(np.float16)

    return (
        tile6(Ac),
        tile6(Wvp),
        tile6(Wo.astype(np.float64)),
        bvwo.astype(np.float32),
    )


def kernel(encoder_outputs, queries, Wq, Wkv, Wo, ln_gamma, ln_beta):
    x = np.ascontiguousarray(
        np.asarray(encoder_outputs, dtype=np.float32).astype(np.float16)
    )
    queries = np.asarray(queries, dtype=np.float32)
    Wq = np.asarray(Wq, dtype=np.float32)
    Wkv = np.asarray(Wkv, dtype=np.float32)
    Wo_np = np.asarray(Wo, dtype=np.float32)
    gamma = np.asarray(ln_gamma, dtype=np.float32)
    beta = np.asarray(ln_beta, dtype=np.float32)

    ac_t, wv_t, wo_t, bvwo = _fold_weights(queries, Wq, Wkv, Wo_np, gamma, beta)

    nc = _get_program()
    in_maps = [
        {
            "x": x[c * B_LOC:(c + 1) * B_LOC],
            "ac": ac_t,
            "wv": wv_t,
            "wo": wo_t,
            "ident": np.eye(128, dtype=np.float16),
        }
        for c in range(N_CORES)
    ]
    res = run_bass_kernel_spmd(nc, in_maps, list(range(N_CORES)))
    y = np.concatenate([res.results[c]["y"] for c in range(N_CORES)], axis=0)
    y = y.reshape(B, 128, ET, NQ).transpose(0, 3, 2, 1).reshape(B, NQ, DIM)
    return np.ascontiguousarray(y + bvwo[None, None, :]).astype(np.float32)
